# revision 7
# baseline (speedup 1.0000x reference)
"""Trainium2 Bass kernel for DifferentiableSparseHypergraph (topk_masking).

Full computation per batch n:
  x_mean = x[n].mean(T)                      (C, V)
  q = Wq @ x_mean + bq                       (O=32, V)   [1x1 conv == matmul]
  q = q / max(||q||_2 over O, eps)
  H_raw = (q^T @ key_prototypes) / sqrt(O)   (V, M=128)
  topk10 -> softmax over the 10 vals -> scatter back; zeros elsewhere.

Kernel strategy (pure data-parallel over batch, 8 cores x 8 batches):
  * t-mean: TWO add levels (t 64 -> 32 -> 16) before the PE, then 4 fp32
    matmuls per batch (512 free each) accumulate into one [32, 512] PSUM
    tile; a strided DVE reduce (8 -> 1) + ACT bias/scale produce q.
    Level 1 runs on DVE; level 2 on GpSimd for batches 0..last-2 (GpSimd
    is otherwise idle) and on DVE for the last two batches (GpSimd's
    in-order queue would gate the tail).  Two levels halve the PE's fp32
    LOW/HIGH matmul time (~93us busy at one level -> ~50us), which was
    co-bottleneck with the 90us HBM stream and pushed the tail ~9us past
    the last byte.
  * software pipelining: DVE executes in order, so reduces run one batch
    late and score chains two late -- anything waiting on another engine
    sits after later batches' adds or it stalls the add stream (and the
    tick counters that gate x-load DMA issue).
  * top-10 runs scale-invariantly on the RAW score matmul pb (still in
    PSUM): per-row ordering of H = pb * rn (rn > 0) equals ordering of pb,
    so max/match_replace/max finds the 10th-largest threshold without
    waiting for the norm. exp(H) fuses the rn scale into ACT's Exp; the
    output is exp * (pb >= t_k) / sum -- identical to softmax-over-topk
    scattered back.
  * output DMAs issue from the ACT engine's DGE queue (ot is computed on
    ACT, so the issue is same-engine in-order, no sem) -- an out DMA on
    the sync queue head-of-line-blocks every later x-load issue.
  * tail shaping: batches last-1 and last stream in chunks so adds/mms
    overlap arrival; the last batch's final chunk is 0.25 MiB so the
    post-last-byte work is tiny.  The second-to-last pair's score matmuls
    are emitted early (PE idle gap) and its vector chain fills the DVE
    gap while the last mm group finishes.
  * batch 0 streams in 1:3 chunks so the first adds start early; PE
    p-state warm-up matmuls run while it streams (cold PE runs at ~2.5x
    cost for the first ~3us).
"""

import numpy as np

import concourse.bacc as bacc
import concourse.bass as bass
import concourse.mybir as mybir
import concourse.tile as tile

N, C, T, V = 64, 256, 64, 64
INTER = 32          # conv out channels
M = 128             # num hyperedges
TOPK = 10
NCORES = 8
FP = mybir.dt.float32
NEG_BIG = -1.0e30


def build_nc(nloc: int) -> bass.Bass:
    """Build the per-core Bass program processing `nloc` batches."""
    assert nloc % 2 == 0 and nloc >= 4
    # Bacc (not bare Bass): its compile()/finalize() pipeline splits
    # multi-semaphore waits into InstEventSemaphore pairs — walrus allows
    # at most one sync wait per regular instruction.
    nc = bacc.Bacc(target_bir_lowering=False, debug=False)

    x = nc.dram_tensor("x", (nloc, C, T, V), FP, kind="ExternalInput")
    wqt = nc.dram_tensor("wqt", (C, INTER), FP, kind="ExternalInput")
    kp = nc.dram_tensor("kp", (INTER, M), FP, kind="ExternalInput")
    bq = nc.dram_tensor("bq", (INTER, 1), FP, kind="ExternalInput")
    out = nc.dram_tensor("out", (nloc, V, M), FP, kind="ExternalOutput")

    A = mybir.AluOpType
    AF = mybir.ActivationFunctionType
    from concourse.tile import add_dep_helper

    last = nloc - 1

    with tile.TileContext(nc) as tc:
        with (
            tc.tile_pool(name="consts", bufs=1) as consts,
            tc.tile_pool(name="xph", bufs=4) as xph,
            tc.tile_pool(name="xp", bufs=2) as xp,
            tc.tile_pool(name="a2p", bufs=2) as a2p,
            tc.tile_pool(name="small", bufs=2) as small,
            tc.tile_pool(name="psA", bufs=3, space="PSUM") as psA,
            tc.tile_pool(name="psB", bufs=2, space="PSUM") as psB,
            tc.tile_pool(name="psS", bufs=1, space="PSUM") as psS,
        ):
            # --- batch 0's first chunk is the FIRST q1 issue: the stream's
            # start gates everything downstream; the consts (48 KB) ride
            # right behind it on the wire.
            xb0 = xph.tile([128, 2 * T * V], FP, tag="xb", name="xb0")
            for h in range(2):
                nc.sync.dma_start(
                    out=xb0[:, h * 4096 : h * 4096 + 1024],
                    in_=x[0, h * 128 : (h + 1) * 128, 0:16],
                )
            wq_sb = consts.tile([128, 2, INTER], FP)    # [c, c_half, o]
            nc.sync.dma_start(
                out=wq_sb[:], in_=wqt.rearrange("(h c) o -> c h o", h=2)
            )
            kp_sb = consts.tile([INTER, M], FP)
            nc.sync.dma_start(out=kp_sb[:], in_=kp[:])
            # bq / ones / 1.5 packed into one tile: each tiny tile costs a
            # 2KB-aligned SBUF slot and SBUF is full.
            cc = consts.tile([128, 4], FP)
            nc.sync.dma_start(out=cc[0:INTER, 0:1], in_=bq[:])
            bq_sb = cc[0:INTER, 0:1]
            for h in range(2):
                nc.sync.dma_start(
                    out=xb0[:, h * 4096 + 1024 : (h + 1) * 4096],
                    in_=x[0, h * 128 : (h + 1) * 128, 16:64],
                )
            # ones-matmul scale: pc = sum_o qsq * INTER = INTER*||q||^2, so
            # rn = rsqrt(pc) directly (no separate INTER scale anywhere).
            ones_sb = cc[0:INTER, 1:2]
            nc.vector.memset(ones_sb, float(INTER))
            c15 = cc[:, 2:3]
            nc.vector.memset(c15, 1.5)

            # The fp32 self-loading matmul can carry at most ONE semaphore
            # wait (walrus S3_LW_STRUCT limit). Absorb the wq/kp DMA waits
            # with dummy 1x1 matmuls so the first real matmuls only wait on
            # their a2-tile sem.
            scr = psS.tile([32, 512], FP)
            d1 = nc.tensor.matmul(
                scr[0:1, 0:1], wq_sb[:, 0, 0:1], wq_sb[:, 0, 0:1],
                start=True, stop=True,
            )
            d2 = nc.tensor.matmul(
                scr[0:1, 0:1], kp_sb[:, 0:1], kp_sb[:, 0:1],
                start=True, stop=True,
            )
            add_dep_helper(d2.ins, d1.ins, sync=False, reason="pe-wait-absorb order")

            # PE p-state warm-up: the PE clocks 0.65 -> 1.2 -> 2.4 GHz with
            # ~3us of continuous work; cold first-batch matmuls ran at
            # ~2.5x cost and that lag leaked into the DVE tick chain that
            # gates x-load issues.  Burn garbage matmuls while batch 0
            # streams in (PE is idle then anyway).
            warm = consts.tile([128, 512], FP)
            nc.gpsimd.memset(warm[:], 0.0)
            prev = d2
            for wi in range(3):
                wm = nc.tensor.matmul(
                    scr[:, 0:256], wq_sb[:, 0, :], warm[:, 0:256],
                    start=True, stop=True,
                )
                add_dep_helper(
                    wm.ins, prev.ins, sync=False, reason="warmup order"
                )
                prev = wm

            q2 = {}            # pair -> q2 tile
            pending = []       # [(n, pa, tslots)] awaiting reduce/bias
            ready_scores = []  # pairs whose reduce is emitted, score isn't
            state = {"first_mm": None}

            def finish_reduce(n, pa, tslots):
                """Emit reduce + bias for batch n."""
                l = n % 2
                p = n // 2
                qtmp = small.tile([INTER, V], FP, tag="qtmp")
                nc.vector.reduce_sum(
                    out=qtmp[:],
                    in_=pa[:, 0 : tslots * V].rearrange(
                        "o (t v) -> o v t", t=tslots
                    ),
                    axis=mybir.AxisListType.X,
                )
                nc.scalar.activation(
                    q2[p][:, l * V : (l + 1) * V],
                    qtmp[:],
                    AF.Identity,
                    bias=bq_sb,
                    scale=1.0 / T,
                )

            score_mm = {}  # p -> (pb, pc, sc) emitted early

            def score_mms(p):
                """The PE/ACT part of pair p's score: qsq, pb, pc."""
                qsq = small.tile([INTER, 2 * V], FP, tag="qsq")
                nc.scalar.activation(qsq[:], q2[p][:], AF.Square)
                # pb and pc share one PSUM tile (PSUM is 8 banks and full)
                pbc = psB.tile([2 * V, M + 4], FP, tag="pbc")
                pb, pc = pbc[:, 0:M], pbc[:, M : M + 1]
                nc.tensor.matmul(pb, q2[p][:], kp_sb[:], start=True, stop=True)
                nc.tensor.matmul(pc, qsq[:], ones_sb, start=True, stop=True)
                score_mm[p] = (pb, pc)

            def score_rest(p):
                """The DVE/ACT chain of pair p's score + out DMA."""
                pb, pc = score_mm.pop(p)
                # One consolidated scratch tile for all the [2V, small]
                # intermediates: separately-tagged tiny tiles each burn a
                # 2KB-aligned SBUF slot per buf and SBUF is 100% full.
                # cols: 0 k, 1 t1, 2 y0, 3 hh, 4 u1, 5 w1, 6 y1, 7 u2,
                #       8 w2, 9 rn, 10 s, 11 r, 16:24 top8a, 24:32 top8b
                sc = small.tile([2 * V, 32], FP, tag="sc")
                t1, y0, hh = sc[:, 1:2], sc[:, 2:3], sc[:, 3:4]
                u1, w1 = sc[:, 4:5], sc[:, 5:6]
                rn = sc[:, 9:10]
                s, r = sc[:, 10:11], sc[:, 11:12]
                top8a, top8b = sc[:, 16:24], sc[:, 24:32]

                # t_k = 10th largest per row: top8, knock out, top8 again
                nc.vector.max(top8a, pb)
                work = small.tile([2 * V, M], FP, tag="work")
                nc.vector.match_replace(work[:], top8a, pb, NEG_BIG)
                nc.vector.max(top8b, work[:])

                # rn = rsqrt(pc) on DVE (fast-inverse-sqrt + 1 Newton step).
                # ACT's Sqrt lives in a different function table than Exp, so
                # using it costs TWO 1.28us ACT_TABLE_LOADs per pair — the
                # whole rsqrt runs on DVE in ~0.9us instead.
                U32 = mybir.dt.uint32
                I32 = mybir.dt.int32
                # read pc's bits straight from PSUM: one fewer serial DVE
                # op + dep hop on every pair's critical path
                nc.vector.tensor_scalar(
                    t1.bitcast(U32), pc.bitcast(U32), 1, None,
                    op0=A.logical_shift_right,
                )
                # y0bits = 0x5f3759df - t1 (DVE int "arith" rounds through
                # fp32 — ~6 low bits of seed lost, irrelevant: the magic
                # seed is only ~3% accurate anyway and Newton runs in fp32)
                nc.vector.tensor_scalar(
                    y0.bitcast(I32), t1.bitcast(I32), -1, 0x5F3759DF,
                    op0=A.mult, op1=A.add,
                )
                nc.vector.tensor_scalar(hh, pc, -0.5, None, op0=A.mult)
                nc.vector.tensor_mul(u1, y0, y0)
                nc.vector.scalar_tensor_tensor(
                    out=w1, in0=hh, scalar=u1, in1=c15,
                    op0=A.mult, op1=A.add,
                )
                # one Newton step suffices: rn rel err ~0.17%, which only
                # rescales exp()'s argument (|H|<~0.6 -> output err ~1e-3,
                # tolerance is 2e-2); the topk mask never sees rn.
                nc.vector.tensor_mul(rn, y0, w1)

                # e = exp(H) = exp(pb * rn)  (rn fused into ACT's scale);
                # masked softmax without scatter:
                # me = (pb >= t_k) * e; out = me / sum(me)
                e = small.tile([2 * V, M], FP, tag="e")
                nc.scalar.activation(e[:], pb, AF.Exp, scale=rn)
                me = small.tile([2 * V, M], FP, tag="me")
                nc.vector.scalar_tensor_tensor(
                    out=me[:],
                    in0=pb,
                    scalar=sc[:, 25:26],
                    in1=e[:],
                    op0=A.is_ge,
                    op1=A.mult,
                    accum_out=s,
                )
                nc.vector.reciprocal(r, s)
                ot = small.tile([2 * V, M], FP, tag="ot")
                nc.scalar.activation(ot[:], me[:], AF.Copy, scale=r)

                # ACT-queue DMA: same-engine in-order after ot, and keeps
                # the sync queue free for x-load issues.
                nc.scalar.dma_start(
                    out=out[2 * p : 2 * p + 2].rearrange("b v m -> (b v) m"),
                    in_=ot[:],
                )

            def finish_score(p):
                score_mms(p)
                score_rest(p)

            def emit_unit(xb, a1s, a2, h, in_off, in_len, pa, mmst, l2_eng):
                """l1 (pair t,t+1 within chunk) + l2 + mm for one chunk unit
                of `in_len` input columns of half h.  Chunked batches only.
                """
                src = xb[:, h * 4096 + in_off : h * 4096 + in_off + in_len]
                src = src.rearrange("p (t two v) -> p t two v", two=2, v=V)
                o1 = in_off // 2
                l1len = in_len // 2
                dst1 = a1s[h][:, o1 : o1 + l1len]
                nc.vector.tensor_add(
                    dst1.rearrange("p (t v) -> p t v", v=V),
                    src[:, :, 0, :],
                    src[:, :, 1, :],
                )
                o2 = in_off // 4
                l2len = in_len // 4
                dst2 = a2[:, h * 1024 + o2 : h * 1024 + o2 + l2len]
                l2_eng.tensor_add(
                    dst2,
                    a1s[h][:, o1 : o1 + l1len // 2],
                    a1s[h][:, o1 + l1len // 2 : o1 + l1len],
                )
                mm = nc.tensor.matmul(
                    pa[:, 0:l2len],
                    wq_sb[:, h, :],
                    dst2,
                    start=(mmst["idx"] == 0),
                    stop=(mmst["idx"] == mmst["total"] - 1),
                )
                if state["first_mm"] is None:
                    state["first_mm"] = mm
                    add_dep_helper(
                        mm.ins, d2.ins, sync=False,
                        reason="pe-wait-absorb order",
                    )
                mmst["idx"] += 1

            for n in range(nloc):
                # --- x DMA issues for batch n
                if n == 0:
                    xb = xb0
                    bounds = [0, 1024, 4096]
                else:
                    xb = xph.tile([128, 2 * T * V], FP, tag="xb")
                    if n == last:
                        bounds = [0, 2048, 3072, 3584, 4096]
                    elif n == last - 1:
                        bounds = [0, 2048, 4096]
                    else:
                        bounds = None
                    if bounds is None:
                        nc.sync.dma_start(
                            out=xb[:].rearrange(
                                "p (h t v) -> p h t v", h=2, v=V
                            ),
                            in_=x[n].rearrange("(h c) t v -> c h t v", h=2),
                        )
                    else:
                        for ci in range(len(bounds) - 1):
                            lo, hi = bounds[ci], bounds[ci + 1]
                            for h in range(2):
                                nc.sync.dma_start(
                                    out=xb[:, h * 4096 + lo : h * 4096 + hi],
                                    in_=x[n, h * 128 : (h + 1) * 128,
                                          lo // V : hi // V],
                                )

                if n % 2 == 0:
                    q2[n // 2] = small.tile(
                        [INTER, 2 * V], FP, tag="q2", name=f"q2_{n // 2}"
                    )
                a2 = a2p.tile([128, 2048], FP, tag="a2", name=f"a2_{n}")

                # --- adds + matmuls
                if n == 0:
                    # arrival-order units of 1024 cols; mms are 256 wide
                    # (pa's [256:512] stays unwritten; the reduce only
                    # reads [0 : tslots*V])
                    pa = psA.tile([INTER, 512], FP, tag="pa")
                    tslots = 4
                    a1s = [
                        xp.tile([128, T * V // 2], FP, tag=f"a1{h}",
                                name=f"a1c{n}_{h}")
                        for h in range(2)
                    ]
                    mmst = {"idx": 0, "total": 8}
                    for ci in range(len(bounds) - 1):
                        for h in range(2):
                            for j in range((bounds[ci + 1] - bounds[ci]) // 1024):
                                emit_unit(xb, a1s, a2, h,
                                          bounds[ci] + j * 1024, 1024,
                                          pa, mmst, nc.gpsimd)
                elif n == last:
                    pa = psA.tile([INTER, 512], FP, tag="pa")
                    tslots = 8
                    a1s = [
                        xp.tile([128, T * V // 2], FP, tag=f"a1{h}",
                                name=f"a1c{n}_{h}")
                        for h in range(2)
                    ]
                    mmst = {"idx": 0, "total": 8}
                    # score mms for the previous pair ride the PE's idle
                    # gap here; its vector chain comes after the adds.
                    for sp in ready_scores:
                        score_mms(sp)
                    for ci in range(len(bounds) - 1):
                        for h in range(2):
                            emit_unit(xb, a1s, a2, h, bounds[ci],
                                      bounds[ci + 1] - bounds[ci],
                                      pa, mmst, nc.vector)
                        if ci == 0:
                            # previous batch's reduce: its mms are done by
                            # now; DVE has a data-wait gap here.
                            nr, par, ts = pending.pop(0)
                            finish_reduce(nr, par, ts)
                elif n == last - 1:
                    pa = psA.tile([INTER, 512], FP, tag="pa")
                    tslots = 8
                    a1s = [
                        xp.tile([128, T * V // 2], FP, tag=f"a1{h}",
                                name=f"a1c{n}_{h}")
                        for h in range(2)
                    ]
                    mmst = {"idx": 0, "total": 4}
                    for ci in range(len(bounds) - 1):
                        for h in range(2):
                            emit_unit(xb, a1s, a2, h, bounds[ci], 2048,
                                      pa, mmst, nc.vector)
                else:
                    pa = psA.tile([INTER, 512], FP, tag="pa")
                    tslots = 8
                    mm_idx = 0
                    for h in range(2):
                        a1 = xp.tile([128, T * V // 2], FP, tag=f"a1{h}")
                        nc.vector.tensor_add(
                            a1[:],
                            xb[:, h * 4096 : h * 4096 + 2048],
                            xb[:, h * 4096 + 2048 : (h + 1) * 4096],
                        )
                        nc.gpsimd.tensor_add(
                            a2[:, h * 1024 : (h + 1) * 1024],
                            a1[:, 0:1024],
                            a1[:, 1024:2048],
                        )
                        for j in range(2):
                            mm = nc.tensor.matmul(
                                pa[:],
                                wq_sb[:, h, :],
                                a2[:, h * 1024 + j * 512
                                   : h * 1024 + (j + 1) * 512],
                                start=(mm_idx == 0),
                                stop=(mm_idx == 3),
                            )
                            if state["first_mm"] is None:
                                state["first_mm"] = mm
                                add_dep_helper(
                                    mm.ins, d2.ins, sync=False,
                                    reason="pe-wait-absorb order",
                                )
                            mm_idx += 1

                # --- software pipelining: reduces one batch late, scores
                # two late (see docstring).
                pending.append((n, pa, tslots))
                if n == last:
                    # previous pair's vector chain fills the DVE gap while
                    # the last mm group drains.
                    for sp in ready_scores:
                        score_rest(sp)
                    ready_scores = []
                elif len(pending) > 1:
                    for sp in ready_scores:
                        finish_score(sp)
                    ready_scores = []
                    nr, par, ts = pending.pop(0)
                    finish_reduce(nr, par, ts)
                    if nr % 2 == 1:
                        ready_scores.append(nr // 2)

            for nr, par, ts in pending:
                finish_reduce(nr, par, ts)
                if nr % 2 == 1:
                    ready_scores.append(nr // 2)
            for sp in ready_scores:
                finish_score(sp)
    nc.finalize()
    return nc


_NC_CACHE: dict[int, bass.Bass] = {}


def _get_nc(nloc: int) -> bass.Bass:
    if nloc not in _NC_CACHE:
        _NC_CACHE[nloc] = build_nc(nloc)
    return _NC_CACHE[nloc]


def _make_in_maps(x, Wq, bq, key_prototypes, ncores):
    nloc = x.shape[0] // ncores
    wqt = np.ascontiguousarray(np.asarray(Wq, dtype=np.float32).T)
    kpc = np.ascontiguousarray(np.asarray(key_prototypes, dtype=np.float32))
    bqc = np.ascontiguousarray(
        np.asarray(bq, dtype=np.float32).reshape(INTER, 1)
    )
    xc = np.asarray(x, dtype=np.float32)
    return [
        {
            "x": np.ascontiguousarray(xc[i * nloc : (i + 1) * nloc]),
            "wqt": wqt,
            "kp": kpc,
            "bq": bqc,
        }
        for i in range(ncores)
    ]


def run(inputs, trace: bool = False):
    """Run on hardware; returns (full_output, BassKernelResults)."""
    from concourse.bass_utils import run_bass_kernel_spmd

    x = inputs["x"]
    nloc = x.shape[0] // NCORES
    nc = _get_nc(nloc)
    in_maps = _make_in_maps(
        x, inputs["Wq"], inputs["bq"], inputs["key_prototypes"], NCORES
    )
    res = run_bass_kernel_spmd(nc, in_maps, list(range(NCORES)), trace=trace)
    out = np.concatenate([r["out"] for r in res.results], axis=0)
    return out, res


def kernel(**inputs) -> np.ndarray:
    out, _ = run(inputs, trace=False)
    return out


# revision 8
# speedup vs baseline: 1.1618x; 1.1618x over previous
"""Trainium2 Bass kernel for DifferentiableSparseHypergraph (topk_masking).

Full computation per batch n:
  x_mean = x[n].mean(T)                      (C, V)
  q = Wq @ x_mean + bq                       (O=32, V)   [1x1 conv == matmul]
  q = q / max(||q||_2 over O, eps)
  H_raw = (q^T @ key_prototypes) / sqrt(O)   (V, M=128)
  topk10 -> softmax over the 10 vals -> scatter back; zeros elsewhere.

Kernel strategy (pure data-parallel over batch, 8 cores x 8 batches):
  * t-mean: TWO add levels (t 64 -> 32 -> 16) before the PE, then 4 fp32
    matmuls per batch (512 free each) accumulate into one [32, 512] PSUM
    tile; a strided DVE reduce (8 -> 1) + ACT bias/scale produce q.
    Level 1 runs on DVE; level 2 on GpSimd for batches 0..last-2 (GpSimd
    is otherwise idle) and on DVE for the last two batches (GpSimd's
    in-order queue would gate the tail).  Two levels halve the PE's fp32
    LOW/HIGH matmul time (~93us busy at one level -> ~50us), which was
    co-bottleneck with the 90us HBM stream and pushed the tail ~9us past
    the last byte.
  * software pipelining: DVE executes in order, so reduces run one batch
    late and score chains two late -- anything waiting on another engine
    sits after later batches' adds or it stalls the add stream (and the
    tick counters that gate x-load DMA issue).
  * top-10 runs scale-invariantly on the RAW score matmul pb (still in
    PSUM): per-row ordering of H = pb * rn (rn > 0) equals ordering of pb,
    so max/match_replace/max finds the 10th-largest threshold without
    waiting for the norm. exp(H) fuses the rn scale into ACT's Exp; the
    output is exp * (pb >= t_k) / sum -- identical to softmax-over-topk
    scattered back.
  * output DMAs issue from the ACT engine's DGE queue (ot is computed on
    ACT, so the issue is same-engine in-order, no sem) -- an out DMA on
    the sync queue head-of-line-blocks every later x-load issue.
  * tail shaping: batches last-1 and last stream in chunks so adds/mms
    overlap arrival; the last batch's final chunk is 0.25 MiB so the
    post-last-byte work is tiny.  The second-to-last pair's score matmuls
    are emitted early (PE idle gap) and its vector chain fills the DVE
    gap while the last mm group finishes.
  * batch 0 streams in 1:3 chunks so the first adds start early; PE
    p-state warm-up matmuls run while it streams (cold PE runs at ~2.5x
    cost for the first ~3us).
"""

import numpy as np

import concourse.bacc as bacc
import concourse.bass as bass
import concourse.mybir as mybir
import concourse.tile as tile

N, C, T, V = 64, 256, 64, 64
INTER = 32          # conv out channels
M = 128             # num hyperedges
TOPK = 10
NCORES = 8
FP = mybir.dt.float32
NEG_BIG = -1.0e30


def build_nc(nloc: int) -> bass.Bass:
    """Build the per-core Bass program processing `nloc` batches."""
    assert nloc % 2 == 0 and nloc >= 4
    # Bacc (not bare Bass): its compile()/finalize() pipeline splits
    # multi-semaphore waits into InstEventSemaphore pairs — walrus allows
    # at most one sync wait per regular instruction.
    nc = bacc.Bacc(target_bir_lowering=False, debug=False)

    x = nc.dram_tensor("x", (nloc, C, T, V), FP, kind="ExternalInput")
    wqt = nc.dram_tensor("wqt", (C, INTER), FP, kind="ExternalInput")
    kp = nc.dram_tensor("kp", (INTER, M), FP, kind="ExternalInput")
    bq = nc.dram_tensor("bq", (INTER, 1), FP, kind="ExternalInput")
    out = nc.dram_tensor("out", (nloc, V, M), FP, kind="ExternalOutput")

    A = mybir.AluOpType
    AF = mybir.ActivationFunctionType
    from concourse.tile import add_dep_helper

    last = nloc - 1

    with tile.TileContext(nc) as tc:
        with (
            tc.tile_pool(name="consts", bufs=1) as consts,
            tc.tile_pool(name="xph", bufs=4) as xph,
            tc.tile_pool(name="xp", bufs=2) as xp,
            tc.tile_pool(name="a2p", bufs=2) as a2p,
            tc.tile_pool(name="small", bufs=2) as small,
            tc.tile_pool(name="psA", bufs=3, space="PSUM") as psA,
            tc.tile_pool(name="psB", bufs=2, space="PSUM") as psB,
            tc.tile_pool(name="psS", bufs=1, space="PSUM") as psS,
        ):
            # --- batch 0's first chunk is the FIRST q1 issue: the stream's
            # start gates everything downstream; the consts (48 KB) ride
            # right behind it on the wire.
            xb0 = xph.tile([128, 2 * T * V], FP, tag="xb", name="xb0")
            for h in range(2):
                nc.sync.dma_start(
                    out=xb0[:, h * 4096 : h * 4096 + 1024],
                    in_=x[0, h * 128 : (h + 1) * 128, 0:16],
                )
            wq_sb = consts.tile([128, 2, INTER], FP)    # [c, c_half, o]
            nc.sync.dma_start(
                out=wq_sb[:], in_=wqt.rearrange("(h c) o -> c h o", h=2)
            )
            kp_sb = consts.tile([INTER, M], FP)
            nc.sync.dma_start(out=kp_sb[:], in_=kp[:])
            # bq / ones / 1.5 packed into one tile: each tiny tile costs a
            # 2KB-aligned SBUF slot and SBUF is full.
            cc = consts.tile([128, 4], FP)
            nc.sync.dma_start(out=cc[0:INTER, 0:1], in_=bq[:])
            bq_sb = cc[0:INTER, 0:1]
            for h in range(2):
                nc.sync.dma_start(
                    out=xb0[:, h * 4096 + 1024 : (h + 1) * 4096],
                    in_=x[0, h * 128 : (h + 1) * 128, 16:64],
                )
            # ones-matmul scale: pc = sum_o qsq * INTER = INTER*||q||^2, so
            # rn = rsqrt(pc) directly (no separate INTER scale anywhere).
            ones_sb = cc[0:INTER, 1:2]
            nc.vector.memset(ones_sb, float(INTER))
            c15 = cc[:, 2:3]
            nc.vector.memset(c15, 1.5)

            # The fp32 self-loading matmul can carry at most ONE semaphore
            # wait (walrus S3_LW_STRUCT limit). Absorb the wq/kp DMA waits
            # with dummy 1x1 matmuls so the first real matmuls only wait on
            # their a2-tile sem.
            scr = psS.tile([32, 512], FP)
            d1 = nc.tensor.matmul(
                scr[0:1, 0:1], wq_sb[:, 0, 0:1], wq_sb[:, 0, 0:1],
                start=True, stop=True,
            )
            d2 = nc.tensor.matmul(
                scr[0:1, 0:1], kp_sb[:, 0:1], kp_sb[:, 0:1],
                start=True, stop=True,
            )
            add_dep_helper(d2.ins, d1.ins, sync=False, reason="pe-wait-absorb order")

            # PE p-state warm-up: the PE clocks 0.65 -> 1.2 -> 2.4 GHz with
            # ~3us of continuous work; cold first-batch matmuls ran at
            # ~2.5x cost and that lag leaked into the DVE tick chain that
            # gates x-load issues.  Burn garbage matmuls while batch 0
            # streams in (PE is idle then anyway).
            warm = consts.tile([128, 512], FP)
            nc.gpsimd.memset(warm[:], 0.0)
            prev = d2
            for wi in range(3):
                wm = nc.tensor.matmul(
                    scr[:, 0:256], wq_sb[:, 0, :], warm[:, 0:256],
                    start=True, stop=True,
                )
                add_dep_helper(
                    wm.ins, prev.ins, sync=False, reason="warmup order"
                )
                prev = wm

            q2 = {}            # pair -> q2 tile
            pending = []       # [(n, pa, tslots)] awaiting reduce/bias
            ready_scores = []  # pairs whose reduce is emitted, score isn't
            state = {"first_mm": None}

            def finish_reduce(n, pa, tslots):
                """Emit reduce + bias for batch n."""
                l = n % 2
                p = n // 2
                qtmp = small.tile([INTER, V], FP, tag="qtmp")
                nc.vector.reduce_sum(
                    out=qtmp[:],
                    in_=pa[:, 0 : tslots * V].rearrange(
                        "o (t v) -> o v t", t=tslots
                    ),
                    axis=mybir.AxisListType.X,
                )
                nc.scalar.activation(
                    q2[p][:, l * V : (l + 1) * V],
                    qtmp[:],
                    AF.Identity,
                    bias=bq_sb,
                    scale=1.0 / T,
                )

            score_mm = {}  # p -> (pb, pc, sc) emitted early

            def score_mms(p):
                """The PE/ACT part of pair p's score: qsq, pb, pc."""
                qsq = small.tile([INTER, 2 * V], FP, tag="qsq")
                nc.scalar.activation(qsq[:], q2[p][:], AF.Square)
                # pb and pc share one PSUM tile (PSUM is 8 banks and full)
                pbc = psB.tile([2 * V, M + 4], FP, tag="pbc")
                pb, pc = pbc[:, 0:M], pbc[:, M : M + 1]
                nc.tensor.matmul(pb, q2[p][:], kp_sb[:], start=True, stop=True)
                nc.tensor.matmul(pc, qsq[:], ones_sb, start=True, stop=True)
                score_mm[p] = (pb, pc)

            def score_rest(p):
                """The DVE/ACT chain of pair p's score + out DMA."""
                pb, pc = score_mm.pop(p)
                # One consolidated scratch tile for all the [2V, small]
                # intermediates: separately-tagged tiny tiles each burn a
                # 2KB-aligned SBUF slot per buf and SBUF is 100% full.
                # cols: 0 k, 1 t1, 2 y0, 3 hh, 4 u1, 5 w1, 6 y1, 7 u2,
                #       8 w2, 9 rn, 10 s, 11 r, 16:24 top8a, 24:32 top8b
                sc = small.tile([2 * V, 32], FP, tag="sc")
                t1, y0, hh = sc[:, 1:2], sc[:, 2:3], sc[:, 3:4]
                u1, w1 = sc[:, 4:5], sc[:, 5:6]
                rn = sc[:, 9:10]
                s, r = sc[:, 10:11], sc[:, 11:12]
                top8a, top8b = sc[:, 16:24], sc[:, 24:32]

                # t_k = 10th largest per row: top8, knock out, top8 again
                nc.vector.max(top8a, pb)
                work = small.tile([2 * V, M], FP, tag="work")
                nc.vector.match_replace(work[:], top8a, pb, NEG_BIG)
                nc.vector.max(top8b, work[:])

                # rn = rsqrt(pc) on DVE (fast-inverse-sqrt + 1 Newton step).
                # ACT's Sqrt lives in a different function table than Exp, so
                # using it costs TWO 1.28us ACT_TABLE_LOADs per pair — the
                # whole rsqrt runs on DVE in ~0.9us instead.
                U32 = mybir.dt.uint32
                I32 = mybir.dt.int32
                # read pc's bits straight from PSUM: one fewer serial DVE
                # op + dep hop on every pair's critical path
                nc.vector.tensor_scalar(
                    t1.bitcast(U32), pc.bitcast(U32), 1, None,
                    op0=A.logical_shift_right,
                )
                # y0bits = 0x5f3759df - t1 (DVE int "arith" rounds through
                # fp32 — ~6 low bits of seed lost, irrelevant: the magic
                # seed is only ~3% accurate anyway and Newton runs in fp32)
                nc.vector.tensor_scalar(
                    y0.bitcast(I32), t1.bitcast(I32), -1, 0x5F3759DF,
                    op0=A.mult, op1=A.add,
                )
                nc.vector.tensor_scalar(hh, pc, -0.5, None, op0=A.mult)
                nc.vector.tensor_mul(u1, y0, y0)
                nc.vector.scalar_tensor_tensor(
                    out=w1, in0=hh, scalar=u1, in1=c15,
                    op0=A.mult, op1=A.add,
                )
                # one Newton step suffices: rn rel err ~0.17%, which only
                # rescales exp()'s argument (|H|<~0.6 -> output err ~1e-3,
                # tolerance is 2e-2); the topk mask never sees rn.
                nc.vector.tensor_mul(rn, y0, w1)

                # e = exp(H) = exp(pb * rn)  (rn fused into ACT's scale);
                # masked softmax without scatter:
                # me = (pb >= t_k) * e; out = me / sum(me)
                e = small.tile([2 * V, M], FP, tag="e")
                nc.scalar.activation(e[:], pb, AF.Exp, scale=rn)
                me = small.tile([2 * V, M], FP, tag="me")
                nc.vector.scalar_tensor_tensor(
                    out=me[:],
                    in0=pb,
                    scalar=sc[:, 25:26],
                    in1=e[:],
                    op0=A.is_ge,
                    op1=A.mult,
                    accum_out=s,
                )
                nc.vector.reciprocal(r, s)
                ot = small.tile([2 * V, M], FP, tag="ot")
                nc.scalar.activation(ot[:], me[:], AF.Copy, scale=r)

                # ACT-queue DMA: same-engine in-order after ot, and keeps
                # the sync queue free for x-load issues.
                nc.scalar.dma_start(
                    out=out[2 * p : 2 * p + 2].rearrange("b v m -> (b v) m"),
                    in_=ot[:],
                )

            def finish_score(p):
                score_mms(p)
                score_rest(p)

            def emit_unit(xb, a1s, a2, h, in_off, in_len, pa, mmst, l2_eng):
                """l1 (pair t,t+1 within chunk) + l2 + mm for one chunk unit
                of `in_len` input columns of half h.  Chunked batches only.
                """
                src = xb[:, h * 4096 + in_off : h * 4096 + in_off + in_len]
                src = src.rearrange("p (t two v) -> p t two v", two=2, v=V)
                o1 = in_off // 2
                l1len = in_len // 2
                dst1 = a1s[h][:, o1 : o1 + l1len]
                nc.vector.tensor_add(
                    dst1.rearrange("p (t v) -> p t v", v=V),
                    src[:, :, 0, :],
                    src[:, :, 1, :],
                )
                o2 = in_off // 4
                l2len = in_len // 4
                dst2 = a2[:, h * 1024 + o2 : h * 1024 + o2 + l2len]
                l2_eng.tensor_add(
                    dst2,
                    a1s[h][:, o1 : o1 + l1len // 2],
                    a1s[h][:, o1 + l1len // 2 : o1 + l1len],
                )
                mm = nc.tensor.matmul(
                    pa[:, 0:l2len],
                    wq_sb[:, h, :],
                    dst2,
                    start=(mmst["idx"] == 0),
                    stop=(mmst["idx"] == mmst["total"] - 1),
                )
                if state["first_mm"] is None:
                    state["first_mm"] = mm
                    add_dep_helper(
                        mm.ins, d2.ins, sync=False,
                        reason="pe-wait-absorb order",
                    )
                mmst["idx"] += 1

            for n in range(nloc):
                # --- x DMA issues for batch n
                if n == 0:
                    xb = xb0
                    bounds = [0, 1024, 4096]
                else:
                    xb = xph.tile([128, 2 * T * V], FP, tag="xb")
                    if n == last:
                        bounds = [0, 2048, 3072, 3584, 4096]
                    elif n == last - 1:
                        bounds = [0, 2048, 4096]
                    else:
                        bounds = None
                    if bounds is None:
                        nc.sync.dma_start(
                            out=xb[:].rearrange(
                                "p (h t v) -> p h t v", h=2, v=V
                            ),
                            in_=x[n].rearrange("(h c) t v -> c h t v", h=2),
                        )
                    else:
                        for ci in range(len(bounds) - 1):
                            lo, hi = bounds[ci], bounds[ci + 1]
                            for h in range(2):
                                nc.sync.dma_start(
                                    out=xb[:, h * 4096 + lo : h * 4096 + hi],
                                    in_=x[n, h * 128 : (h + 1) * 128,
                                          lo // V : hi // V],
                                )

                if n % 2 == 0:
                    q2[n // 2] = small.tile(
                        [INTER, 2 * V], FP, tag="q2", name=f"q2_{n // 2}"
                    )
                a2 = a2p.tile([128, 2048], FP, tag="a2", name=f"a2_{n}")

                # --- adds + matmuls
                if n == 0:
                    # arrival-order units of 1024 cols; mms are 256 wide
                    # (pa's [256:512] stays unwritten; the reduce only
                    # reads [0 : tslots*V])
                    pa = psA.tile([INTER, 512], FP, tag="pa")
                    tslots = 4
                    a1s = [
                        xp.tile([128, T * V // 2], FP, tag=f"a1{h}",
                                name=f"a1c{n}_{h}")
                        for h in range(2)
                    ]
                    mmst = {"idx": 0, "total": 8}
                    for ci in range(len(bounds) - 1):
                        for h in range(2):
                            for j in range((bounds[ci + 1] - bounds[ci]) // 1024):
                                emit_unit(xb, a1s, a2, h,
                                          bounds[ci] + j * 1024, 1024,
                                          pa, mmst, nc.vector)
                elif n == last:
                    pa = psA.tile([INTER, 512], FP, tag="pa")
                    tslots = 8
                    a1s = [
                        xp.tile([128, T * V // 2], FP, tag=f"a1{h}",
                                name=f"a1c{n}_{h}")
                        for h in range(2)
                    ]
                    mmst = {"idx": 0, "total": 8}
                    # score mms for the previous pair ride the PE's idle
                    # gap here; its vector chain comes after the adds.
                    for sp in ready_scores:
                        score_mms(sp)
                    for ci in range(len(bounds) - 1):
                        for h in range(2):
                            emit_unit(xb, a1s, a2, h, bounds[ci],
                                      bounds[ci + 1] - bounds[ci],
                                      pa, mmst, nc.vector)
                        if ci == 0:
                            # previous batch's reduce: its mms are done by
                            # now; DVE has a data-wait gap here.
                            nr, par, ts = pending.pop(0)
                            finish_reduce(nr, par, ts)
                elif n == last - 1:
                    pa = psA.tile([INTER, 512], FP, tag="pa")
                    tslots = 8
                    a1s = [
                        xp.tile([128, T * V // 2], FP, tag=f"a1{h}",
                                name=f"a1c{n}_{h}")
                        for h in range(2)
                    ]
                    mmst = {"idx": 0, "total": 4}
                    for ci in range(len(bounds) - 1):
                        for h in range(2):
                            emit_unit(xb, a1s, a2, h, bounds[ci], 2048,
                                      pa, mmst, nc.vector)
                else:
                    pa = psA.tile([INTER, 512], FP, tag="pa")
                    tslots = 8
                    mm_idx = 0
                    for h in range(2):
                        a1 = xp.tile([128, T * V // 2], FP, tag=f"a1{h}")
                        nc.vector.tensor_add(
                            a1[:],
                            xb[:, h * 4096 : h * 4096 + 2048],
                            xb[:, h * 4096 + 2048 : (h + 1) * 4096],
                        )
                        nc.vector.tensor_add(
                            a2[:, h * 1024 : (h + 1) * 1024],
                            a1[:, 0:1024],
                            a1[:, 1024:2048],
                        )
                        for j in range(2):
                            mm = nc.tensor.matmul(
                                pa[:],
                                wq_sb[:, h, :],
                                a2[:, h * 1024 + j * 512
                                   : h * 1024 + (j + 1) * 512],
                                start=(mm_idx == 0),
                                stop=(mm_idx == 3),
                            )
                            if state["first_mm"] is None:
                                state["first_mm"] = mm
                                add_dep_helper(
                                    mm.ins, d2.ins, sync=False,
                                    reason="pe-wait-absorb order",
                                )
                            mm_idx += 1

                # --- software pipelining: reduces one batch late, scores
                # two late (see docstring).
                pending.append((n, pa, tslots))
                if n == last:
                    # previous pair's vector chain fills the DVE gap while
                    # the last mm group drains.
                    for sp in ready_scores:
                        score_rest(sp)
                    ready_scores = []
                elif len(pending) > 1:
                    for sp in ready_scores:
                        finish_score(sp)
                    ready_scores = []
                    nr, par, ts = pending.pop(0)
                    finish_reduce(nr, par, ts)
                    if nr % 2 == 1:
                        ready_scores.append(nr // 2)

            for nr, par, ts in pending:
                finish_reduce(nr, par, ts)
                if nr % 2 == 1:
                    ready_scores.append(nr // 2)
            for sp in ready_scores:
                finish_score(sp)
    nc.finalize()
    return nc


_NC_CACHE: dict[int, bass.Bass] = {}


def _get_nc(nloc: int) -> bass.Bass:
    if nloc not in _NC_CACHE:
        _NC_CACHE[nloc] = build_nc(nloc)
    return _NC_CACHE[nloc]


def _make_in_maps(x, Wq, bq, key_prototypes, ncores):
    nloc = x.shape[0] // ncores
    wqt = np.ascontiguousarray(np.asarray(Wq, dtype=np.float32).T)
    kpc = np.ascontiguousarray(np.asarray(key_prototypes, dtype=np.float32))
    bqc = np.ascontiguousarray(
        np.asarray(bq, dtype=np.float32).reshape(INTER, 1)
    )
    xc = np.asarray(x, dtype=np.float32)
    return [
        {
            "x": np.ascontiguousarray(xc[i * nloc : (i + 1) * nloc]),
            "wqt": wqt,
            "kp": kpc,
            "bq": bqc,
        }
        for i in range(ncores)
    ]


def run(inputs, trace: bool = False):
    """Run on hardware; returns (full_output, BassKernelResults)."""
    from concourse.bass_utils import run_bass_kernel_spmd

    x = inputs["x"]
    nloc = x.shape[0] // NCORES
    nc = _get_nc(nloc)
    in_maps = _make_in_maps(
        x, inputs["Wq"], inputs["bq"], inputs["key_prototypes"], NCORES
    )
    res = run_bass_kernel_spmd(nc, in_maps, list(range(NCORES)), trace=trace)
    out = np.concatenate([r["out"] for r in res.results], axis=0)
    return out, res


def kernel(**inputs) -> np.ndarray:
    out, _ = run(inputs, trace=False)
    return out
